# revision 36
# baseline (speedup 1.0000x reference)
"""EvoAttn (V-only causal self-attention) on 8 Trainium2 NeuronCores.

Full input x:(2,2048,2048) fp32 -> full output (2,2048,2048) fp32.
Sharding: 32 (b,h) head-slices, 4 per core (head parallel).

Per (b,h) on-device (L=2048, D=128), with V = x[b,:,h*128:(h+1)*128]:
  S^T tiles  : psum[k=128, q<=1024] = VT[:,kblk].T @ VT[:,qchunk]  (bf16 PE)
  E^T tiles  : exp(S^T/sqrt(D)) split across TWO engines:
                 - ACT: exp activation (PSUM->SBUF bf16)
                 - DVE: custom op EVO_SCHRA_EXP: int16(S*a+b) bit-viewed as
                   bf16 ~= 2^(S*a') (Schraudolph), causal mask fused via
                   select(Idx >= k_partition_iota)
  causal mask: ACT-produced diagonal tiles multiply a host 0/1 mask (DVE)
  PV         : psum[q=128, 129] += E^T[:,qsub].T @ Vaug[kblk]  where Vaug
               has a ones column -> col 128 = softmax denominator
  normalize  : custom DVE op EVO_NORM_RECIP: out_bf16 = num * (1/den) with
               a 1-Newton-Raphson bitwise-NOT reciprocal hoisted to the
               instruction's seed stage (zero body cost)
"""

import sys

for _p in ("/opt/trn_rl_repo",):
    if _p not in sys.path:
        sys.path.insert(0, _p)

import numpy as np
import ml_dtypes

BF16 = ml_dtypes.bfloat16

B, L, E = 2, 2048, 2048
H, D = 16, 128
P = 128          # partition dim / k-block
QC = 1024        # q chunk (two PSUM banks of fp32; one exp per chunk-tile)
NKB = L // P     # 16 k-blocks
NQC = L // QC    # 2 q chunks
QB = QC // P     # 8 q-blocks (PV granularity) per chunk
NCORES = 8
H4 = (B * H) // NCORES  # 4 heads per core
SCALE = 1.0 / float(np.sqrt(D))

# Schraudolph bitcast-exp constants (int16 bits viewed as bf16):
#   i16 = round(s * SA + SB);  bitcast(i16) ~= exp(s * SCALE)
LOG2E = float(np.log2(np.e))
SA = SCALE * LOG2E * 128.0
SB = 127.0 * 128.0 - 0.043 * 128.0
# 1-NR reciprocal (bitwise-NOT seed) minimax constants
NR_C0 = -0.23551975
NR_C1 = 2.00173669

# exp routing balance (ns-model): engine rate per free column + per-inst cost
ACT_RATE, ACT_OVH = 0.8333, 280.0
DVE_RATE, DVE_OVH = 1.0417, 190.0
NORM_COST = 300.0
DVE_BIAS = 800.0  # positive pushes more work to ACT

_cache = {}


def _register_dve_ops():
    """Append the two custom DVE ops to the concourse registry (idempotent).
    uops_sha is computed at runtime with the same construction
    DveOp.compile uses, so table generation is self-consistent."""
    from concourse import dve_ops
    from concourse.dve_spec import (
        Spec, Src0, C0, C1, C2, Zero, One, AluOp, Bin, Idx, select, lower,
    )
    from concourse.dve_uop import DveOpSpec

    if any(o.name == "EVO_SCHRA_EXP" for o in dve_ops.OPS):
        from concourse.dve_ops import OPS
        return {o.name: o for o in OPS
                if o.name in ("EVO_SCHRA_EXP", "EVO_NORM_RECIP")}

    def _schra_ref(in0, in1, s0, s1, imm2):
        thr = np.asarray(s0, dtype=np.float32).reshape(-1, 1)
        idx = np.arange(in0.shape[-1], dtype=np.float32)[None, :]
        return np.where(idx >= thr, in0 * s1 + imm2, 0.0).astype(np.float32)

    schra_spec = Spec(
        body=select(Idx >= C0, Src0 * C1 + C2, Zero),
        reference=_schra_ref,
    )

    _c = C0 * One
    _not = Bin(AluOp.BITWISE_NOT, _c, _c)
    _y0 = _not * C1
    _y1 = _y0 * (C2 - C0 * _y0)

    def _norm_ref(in0, in1, s0, s1, imm2):
        s0a = np.asarray(s0, dtype=np.float32).reshape(-1, 1)
        nx = (~s0a.view(np.int32)).view(np.float32)
        y0 = nx * s1
        y1 = y0 * (imm2 - s0a * y0)
        return (np.asarray(in0, np.float32) * y1).astype(np.float32)

    norm_spec = Spec(body=Src0 * _y1, reference=_norm_ref)

    ops = {}
    for name, spec in (("EVO_SCHRA_EXP", schra_spec),
                       ("EVO_NORM_RECIP", norm_spec)):
        shas = {}
        for ver in ("v3", "v4"):
            uops = lower(spec, ver=ver)
            shas[ver] = DveOpSpec(name=name, opcode=0, uops=uops,
                                  rd1_en=False).sha(ver)
        op = dve_ops.DveOp(name, spec, subdim=False, uops_sha=shas)
        dve_ops.OPS.append(op)
        dve_ops._SUB_OPCODE_FOR_NAME[name] = (
            dve_ops._CUSTOM_DVE_ROW_BASE + len(dve_ops.OPS) - 1)
        ops[name] = op
    return ops


def _build_nc():
    import concourse.bacc as bacc
    import concourse.mybir as mybir
    import concourse.tile as tile
    from contextlib import ExitStack

    ops = _register_dve_ops()
    SCHRA_OP = ops["EVO_SCHRA_EXP"]
    NORM_OP = ops["EVO_NORM_RECIP"]

    f32 = mybir.dt.float32
    bf16 = mybir.dt.bfloat16
    i16 = mybir.dt.int16

    nc = bacc.Bacc("TRN2", target_bir_lowering=False, debug=False,
                   num_devices=NCORES)

    # DRAM I/O (per-core shapes); inputs are chunk-major so every DMA
    # chunk is one fully contiguous DRAM block (maximal packets)
    x_vq = nc.dram_tensor("vq", [H4, 2, P, QB * (D + 1)], bf16,
                          kind="ExternalInput")
    x_vt = nc.dram_tensor("vt", [H4, 4, D, 512], bf16, kind="ExternalInput")
    x_mask = nc.dram_tensor("mask", [P, P], bf16, kind="ExternalInput")
    x_kio = nc.dram_tensor("kio", [P, 1], f32, kind="ExternalInput")
    y = nc.dram_tensor("y", [H4, L, D], bf16, kind="ExternalOutput")

    EXP = mybir.ActivationFunctionType.Exp

    with tile.TileContext(nc) as tc, ExitStack() as ctx:
        const_pool = ctx.enter_context(tc.tile_pool(name="const", bufs=1))
        vq_pool = ctx.enter_context(tc.tile_pool(name="vq", bufs=2))
        vt_pool = ctx.enter_context(tc.tile_pool(name="vt", bufs=2))
        pt_pool = ctx.enter_context(tc.tile_pool(name="pt", bufs=48))
        out_pool = ctx.enter_context(tc.tile_pool(name="out", bufs=2))
        ps_s = ctx.enter_context(tc.tile_pool(name="ps_s", bufs=3, space="PSUM"))
        ps_o = ctx.enter_context(tc.tile_pool(name="ps_o", bufs=2, space="PSUM"))

        mask_t = const_pool.tile([P, P], bf16)
        nc.gpsimd.dma_start(mask_t[:], x_mask[:, :])
        kio_t = const_pool.tile([P, 1], f32)
        nc.gpsimd.dma_start(kio_t[:], x_kio[:, :])
        # warm the ACT exp table now (1.3us table load otherwise lands on
        # the first real exp's critical path); scale=0 makes the
        # uninitialized input irrelevant
        warm_t = const_pool.tile([P, 1], bf16)
        nc.scalar.activation(warm_t[:], warm_t[:], EXP, scale=0.0)


        # per-head state created lazily by the flat tile stream
        vt_ts, vq_ts, o_bigs, pt_tiles = {}, {}, {}, {}

        # engine load balancer state (ns model)
        load = {"act": 0.0, "dve": 0.0}

        def load_head(h):
            # rebalance locally per head: a global tally lets one engine
            # run time-locally hot even when cumulative loads match
            m = min(load["act"], load["dve"])
            load["act"] -= m
            load["dve"] -= m
            vt_t = vt_pool.tile([P, L], bf16, tag="vt")
            vq_t = vq_pool.tile([P, NKB, D + 1], bf16, tag="vq")
            if h == 0:
                # cold start: land the first 256 columns on their own so
                # the first (split) scores matmul fires earlier
                nc.sync.dma_start(vt_t[:, 0:256], x_vt[h][0][:, 0:256])
                nc.sync.dma_start(vt_t[:, 256:512], x_vt[h][0][:, 256:512])
                for c in range(1, 4):
                    nc.sync.dma_start(vt_t[:, c * 512:(c + 1) * 512],
                                      x_vt[h][c])
            else:
                for c in range(4):  # vt first: the first scores tile needs it
                    nc.sync.dma_start(vt_t[:, c * 512:(c + 1) * 512],
                                      x_vt[h][c])
            for c in range(2):
                nc.sync.dma_start(
                    vq_t[:, c * QB:(c + 1) * QB, :],
                    x_vq[h][c].rearrange("p (kb c) -> p kb c", kb=QB),
                )
            vt_ts[h], vq_ts[h] = vt_t, vq_t
            o_bigs[h] = out_pool.tile([P, NKB, D], bf16, tag="obig",
                                      name="obig")

        pending_norms = []  # (h, qi) finished chains awaiting normalize

        def flush_norms():
            while pending_norms:
                h, qi, po = pending_norms.pop(0)
                nc.vector._custom_dve(
                    NORM_OP,
                    out=o_bigs[h][:, qi, :],
                    in0=po[:, :D],
                    s0=po[:, D:D + 1],
                    s1=NR_C0,
                    imm2=NR_C1,
                )
                load["dve"] += NORM_COST
                if h == H4 - 1 and qi <= 11:
                    # last head: drain early rows per-qi via Sync HWDGE
                    # (idle there) so only the last quarter remains at the
                    # end
                    nc.sync.dma_start(y[h][qi * P:(qi + 1) * P, :],
                                      o_bigs[h][:, qi, :])
                elif h == H4 - 1 and qi == 13:
                    # final quarter leaves in overlapping pieces on two DMA
                    # queues: rows 12-13 as soon as their norms land, then
                    # 14 and 15 in parallel on separate queues
                    nc.sync.dma_start(
                        y[h][1536:1792, :].rearrange("(kb p) d -> p kb d",
                                                     p=P),
                        o_bigs[h][:, 12:14, :],
                    )
                elif h == H4 - 1 and qi == 14:
                    nc.gpsimd.dma_start(y[h][1792:1920, :],
                                        o_bigs[h][:, 14, :])
                elif h == H4 - 1 and qi == 15:
                    nc.sync.dma_start(y[h][1920:2048, :],
                                      o_bigs[h][:, 15, :])
                elif qi % 4 == 3:  # finished an output quarter -> drain it
                    q4 = qi // 4
                    nc.gpsimd.dma_start(
                        y[h][q4 * 512:(q4 + 1) * 512, :].rearrange(
                            "(kb p) d -> p kb d", p=P),
                        o_bigs[h][:, q4 * 4:(q4 + 1) * 4, :],
                    )

        def emit_scores_tile(h, qc, kb):
            j = kb - QB * qc  # >=0 -> diagonal-chunk block
            off = max(0, j) * P
            vt_t = vt_ts[h]
            q0 = qc * QC
            ps = ps_s.tile([P, QC], f32, tag="ps_s")
            if off < 512:
                if h == 0 and qc == 0 and kb == 0:
                    # split the very first matmul so it only waits on the
                    # first half of the cold-start vt DMA
                    nc.tensor.matmul(ps[:, 0:256], vt_t[:, 0:P],
                                     vt_t[:, 0:256], start=True, stop=True)
                    nc.tensor.matmul(ps[:, 256:512], vt_t[:, 0:P],
                                     vt_t[:, 256:512], start=True, stop=True)
                else:
                    nc.tensor.matmul(ps[:, off:512],
                                     vt_t[:, kb * P:(kb + 1) * P],
                                     vt_t[:, q0 + off:q0 + 512],
                                     start=True, stop=True)
                nc.tensor.matmul(ps[:, 512:],
                                 vt_t[:, kb * P:(kb + 1) * P],
                                 vt_t[:, q0 + 512:q0 + QC],
                                 start=True, stop=True)
            else:
                nc.tensor.matmul(ps[:, off:],
                                 vt_t[:, kb * P:(kb + 1) * P],
                                 vt_t[:, q0 + off:q0 + QC],
                                 start=True, stop=True)
            pt = pt_pool.tile([P, QC], bf16, tag="pt")
            cols = QC - off
            cost_a = cols * ACT_RATE + ACT_OVH
            cost_d = cols * DVE_RATE + DVE_OVH
            # on the last head DVE still owns the epilogue norms while ACT
            # goes idle, so shift exp work toward ACT there
            bias = DVE_BIAS + (1600.0 if h == H4 - 1 else 0.0)
            use_act = load["act"] + cost_a <= load["dve"] + cost_d + bias
            if use_act:
                load["act"] += cost_a
                nc.scalar.activation(pt[:, off:], ps[:, off:], EXP,
                                     scale=SCALE)
                if j >= 0:
                    # mask on GPSIMD: keeps the DVE queue free for exp+norm
                    nc.gpsimd.tensor_mul(pt[:, off:off + P],
                                         pt[:, off:off + P], mask_t[:])
            else:
                load["dve"] += cost_d
                nc.vector._custom_dve(
                    SCHRA_OP,
                    out=pt[:, off:].bitcast(i16),
                    in0=ps[:, off:],
                    s0=(kio_t[:] if j >= 0 else 0.0),
                    s1=SA,
                    imm2=SB,
                )
            flush_norms()
            pt_tiles[(h, qc, kb)] = pt

        # PV chain work is drained as individual matmuls from a FIFO so
        # each scores tile is followed by just enough PV matmuls to fill
        # PE's slack while the exp engines run. A backlog floor keeps work
        # in reserve for the chain-less pass-1-early windows.
        chain_fifo = []   # (h, qc, qi) in completion order
        cur = {"mm": 0, "po": None}   # cursor into chain_fifo[0]
        backlog = {"mms": 0}
        done = {"scores": False}

        def emit_chain_mms(n):
            while n > 0 and chain_fifo:
                h, qc, qi = chain_fifo[0]
                qsub = qi - QB * qc
                if cur["po"] is None:
                    cur["po"] = ps_o.tile([P, D + 1], f32, tag="ps_o",
                                          name="po")
                    cur["mm"] = 0
                kb = cur["mm"]
                nc.tensor.matmul(
                    cur["po"][:],
                    pt_tiles[(h, qc, kb)][:, qsub * P:(qsub + 1) * P],
                    vq_ts[h][:, kb, :],
                    start=(kb == 0), stop=(kb == qi),
                )
                cur["mm"] += 1
                backlog["mms"] -= 1
                n -= 1
                if cur["mm"] == qi + 1:
                    pending_norms.append((h, qi, cur["po"]))
                    chain_fifo.pop(0)
                    cur["po"] = None
                    # in the epilogue there are no more scores tiles to
                    # piggyback on: emit norms as chains complete so the
                    # output drains overlap the remaining PV matmuls
                    if done["scores"]:
                        flush_norms()

        # chain matmuls run ~45% faster in uninterrupted streaks (weight
        # loads pipeline), so accumulate the per-tile budget and emit in
        # bursts of ~2 tiles' worth
        FLOOR = 56
        BURST = 12
        budget_acc = {"n": 0}
        for h in range(H4):
            load_head(h)
            for qc in range(NQC):
                for kb in range(QB * qc + QB):
                    j = kb - QB * qc
                    # the reserve exists FOR the chain-less pass-1-early
                    # window: release it there (and on the last head),
                    # hold it during append windows
                    in_p1_early = qc == NQC - 1 and j < 0
                    floor = 0 if (h == H4 - 1 or in_p1_early) else FLOOR
                    emit_scores_tile(h, qc, kb)
                    if j >= 0:
                        qi = QB * qc + j
                        chain_fifo.append((h, qc, qi))
                        backlog["mms"] += qi + 1
                    cols = QC - max(0, j) * P
                    budget_acc["n"] += cols // 190 + 2
                    if h == H4 - 1 and qc == NQC - 1:
                        # drain the backlog during the final chunk so the
                        # last chains don't trail the last scores tile
                        budget_acc["n"] += 6
                    if budget_acc["n"] >= BURST:
                        emit_chain_mms(
                            min(budget_acc["n"], backlog["mms"] - floor))
                        budget_acc["n"] = 0
        done["scores"] = True
        emit_chain_mms(backlog["mms"])
        flush_norms()

    nc.compile()
    return nc


def _get_nc():
    if "nc" not in _cache:
        _cache["nc"] = _build_nc()
    return _cache["nc"]


def _make_mask():
    # keep (partition=k_local, free=q_local) where q_local >= k_local
    pk = np.arange(P)[:, None]
    fq = np.arange(P)[None, :]
    return (fq >= pk).astype(BF16)


def kernel(x):
    from concourse.bass_utils import run_bass_kernel_spmd

    x = np.asarray(x)
    in_dtype = x.dtype
    assert x.shape == (B, L, E)

    nc = _get_nc()

    # (B, L, H, D) -> (B*H, L, D), bf16
    v = np.ascontiguousarray(
        x.reshape(B, L, H, D).transpose(0, 2, 1, 3)
    ).reshape(B * H, L, D).astype(BF16)

    mask = _make_mask()
    kio = np.arange(P, dtype=np.float32).reshape(P, 1)
    in_maps = []
    for c in range(NCORES):
        sl = v[H4 * c:H4 * (c + 1)]                      # (H4, L, D)
        # chunk-major vq: [H4, 2, P, QB*(D+1)], ones column appended
        vq = np.ones((H4, P, NKB, D + 1), dtype=BF16)
        vq[..., :D] = sl.reshape(H4, NKB, P, D).transpose(0, 2, 1, 3)
        vq = np.ascontiguousarray(
            vq.reshape(H4, P, 2, QB * (D + 1)).transpose(0, 2, 1, 3))
        # chunk-major vt: [H4, 4, D, 512]
        vt = sl.transpose(0, 2, 1).reshape(H4, D, 4, 512)
        vt = np.ascontiguousarray(vt.transpose(0, 2, 1, 3))
        in_maps.append({"vq": vq, "vt": vt, "mask": mask, "kio": kio})

    import os

    kwargs = {}
    if os.environ.get("KERNEL_TRACE"):
        kwargs["trace"] = True
        if os.environ.get("KERNEL_TRACE_DIR"):
            kwargs["tmpdir"] = os.environ["KERNEL_TRACE_DIR"]
    res = run_bass_kernel_spmd(nc, in_maps, core_ids=list(range(NCORES)), **kwargs)
    _cache["last_results"] = res
    ys = np.stack([np.asarray(res.results[c]["y"]) for c in range(NCORES)],
                  axis=0)
    # (NCORES, H4, L, D) -> (B, H, L, D) -> (B, L, E)
    out = ys.astype(np.float32).reshape(B, H, L, D).transpose(
        0, 2, 1, 3).reshape(B, L, E)
    return out.astype(in_dtype, copy=False)


# revision 37
# speedup vs baseline: 1.0086x; 1.0086x over previous
"""EvoAttn (V-only causal self-attention) on 8 Trainium2 NeuronCores.

Full input x:(2,2048,2048) fp32 -> full output (2,2048,2048) fp32.
Sharding: 32 (b,h) head-slices, 4 per core (head parallel).

Per (b,h) on-device (L=2048, D=128), with V = x[b,:,h*128:(h+1)*128]:
  S^T tiles  : psum[k=128, q<=1024] = VT[:,kblk].T @ VT[:,qchunk]  (bf16 PE)
  E^T tiles  : exp(S^T/sqrt(D)) split across TWO engines:
                 - ACT: exp activation (PSUM->SBUF bf16)
                 - DVE: custom op EVO_SCHRA_EXP: int16(S*a+b) bit-viewed as
                   bf16 ~= 2^(S*a') (Schraudolph), causal mask fused via
                   select(Idx >= k_partition_iota)
  causal mask: ACT-produced diagonal tiles multiply a host 0/1 mask (DVE)
  PV         : psum[q=128, 129] += E^T[:,qsub].T @ Vaug[kblk]  where Vaug
               has a ones column -> col 128 = softmax denominator
  normalize  : custom DVE op EVO_NORM_RECIP: out_bf16 = num * (1/den) with
               a 1-Newton-Raphson bitwise-NOT reciprocal hoisted to the
               instruction's seed stage (zero body cost)
"""

import sys

for _p in ("/opt/trn_rl_repo",):
    if _p not in sys.path:
        sys.path.insert(0, _p)

import numpy as np
import ml_dtypes

BF16 = ml_dtypes.bfloat16

B, L, E = 2, 2048, 2048
H, D = 16, 128
P = 128          # partition dim / k-block
QC = 1024        # q chunk (two PSUM banks of fp32; one exp per chunk-tile)
NKB = L // P     # 16 k-blocks
NQC = L // QC    # 2 q chunks
QB = QC // P     # 8 q-blocks (PV granularity) per chunk
NCORES = 8
H4 = (B * H) // NCORES  # 4 heads per core
SCALE = 1.0 / float(np.sqrt(D))

# Schraudolph bitcast-exp constants (int16 bits viewed as bf16):
#   i16 = round(s * SA + SB);  bitcast(i16) ~= exp(s * SCALE)
LOG2E = float(np.log2(np.e))
SA = SCALE * LOG2E * 128.0
SB = 127.0 * 128.0 - 0.043 * 128.0
# 1-NR reciprocal (bitwise-NOT seed) minimax constants
NR_C0 = -0.23551975
NR_C1 = 2.00173669

# exp routing balance (ns-model): engine rate per free column + per-inst cost
ACT_RATE, ACT_OVH = 0.8333, 280.0
DVE_RATE, DVE_OVH = 1.0417, 190.0
NORM_COST = 300.0
DVE_BIAS = 800.0  # positive pushes more work to ACT

_cache = {}


def _register_dve_ops():
    """Append the two custom DVE ops to the concourse registry (idempotent).
    uops_sha is computed at runtime with the same construction
    DveOp.compile uses, so table generation is self-consistent."""
    from concourse import dve_ops
    from concourse.dve_spec import (
        Spec, Src0, C0, C1, C2, Zero, One, AluOp, Bin, Idx, select, lower,
    )
    from concourse.dve_uop import DveOpSpec

    if any(o.name == "EVO_SCHRA_EXP" for o in dve_ops.OPS):
        from concourse.dve_ops import OPS
        return {o.name: o for o in OPS
                if o.name in ("EVO_SCHRA_EXP", "EVO_NORM_RECIP")}

    def _schra_ref(in0, in1, s0, s1, imm2):
        thr = np.asarray(s0, dtype=np.float32).reshape(-1, 1)
        idx = np.arange(in0.shape[-1], dtype=np.float32)[None, :]
        return np.where(idx >= thr, in0 * s1 + imm2, 0.0).astype(np.float32)

    schra_spec = Spec(
        body=select(Idx >= C0, Src0 * C1 + C2, Zero),
        reference=_schra_ref,
    )

    _c = C0 * One
    _not = Bin(AluOp.BITWISE_NOT, _c, _c)
    _y0 = _not * C1
    _y1 = _y0 * (C2 - C0 * _y0)

    def _norm_ref(in0, in1, s0, s1, imm2):
        s0a = np.asarray(s0, dtype=np.float32).reshape(-1, 1)
        nx = (~s0a.view(np.int32)).view(np.float32)
        y0 = nx * s1
        y1 = y0 * (imm2 - s0a * y0)
        return (np.asarray(in0, np.float32) * y1).astype(np.float32)

    norm_spec = Spec(body=Src0 * _y1, reference=_norm_ref)

    ops = {}
    for name, spec in (("EVO_SCHRA_EXP", schra_spec),
                       ("EVO_NORM_RECIP", norm_spec)):
        shas = {}
        for ver in ("v3", "v4"):
            uops = lower(spec, ver=ver)
            shas[ver] = DveOpSpec(name=name, opcode=0, uops=uops,
                                  rd1_en=False).sha(ver)
        op = dve_ops.DveOp(name, spec, subdim=False, uops_sha=shas)
        dve_ops.OPS.append(op)
        dve_ops._SUB_OPCODE_FOR_NAME[name] = (
            dve_ops._CUSTOM_DVE_ROW_BASE + len(dve_ops.OPS) - 1)
        ops[name] = op
    return ops


def _build_nc():
    import concourse.bacc as bacc
    import concourse.mybir as mybir
    import concourse.tile as tile
    from contextlib import ExitStack

    ops = _register_dve_ops()
    SCHRA_OP = ops["EVO_SCHRA_EXP"]
    NORM_OP = ops["EVO_NORM_RECIP"]

    f32 = mybir.dt.float32
    bf16 = mybir.dt.bfloat16
    i16 = mybir.dt.int16

    nc = bacc.Bacc("TRN2", target_bir_lowering=False, debug=False,
                   num_devices=NCORES)

    # DRAM I/O (per-core shapes); inputs are chunk-major so every DMA
    # chunk is one fully contiguous DRAM block (maximal packets)
    x_vq = nc.dram_tensor("vq", [H4, 2, P, QB * (D + 1)], bf16,
                          kind="ExternalInput")
    x_vt = nc.dram_tensor("vt", [H4, 4, D, 512], bf16, kind="ExternalInput")
    x_mask = nc.dram_tensor("mask", [P, P], bf16, kind="ExternalInput")
    x_kio = nc.dram_tensor("kio", [P, 1], f32, kind="ExternalInput")
    y = nc.dram_tensor("y", [H4, L, D], bf16, kind="ExternalOutput")

    EXP = mybir.ActivationFunctionType.Exp

    with tile.TileContext(nc) as tc, ExitStack() as ctx:
        const_pool = ctx.enter_context(tc.tile_pool(name="const", bufs=1))
        vq_pool = ctx.enter_context(tc.tile_pool(name="vq", bufs=2))
        vt_pool = ctx.enter_context(tc.tile_pool(name="vt", bufs=2))
        pt_pool = ctx.enter_context(tc.tile_pool(name="pt", bufs=48))
        out_pool = ctx.enter_context(tc.tile_pool(name="out", bufs=2))
        ps_s = ctx.enter_context(tc.tile_pool(name="ps_s", bufs=3, space="PSUM"))
        ps_o = ctx.enter_context(tc.tile_pool(name="ps_o", bufs=2, space="PSUM"))

        mask_t = const_pool.tile([P, P], bf16)
        nc.gpsimd.dma_start(mask_t[:], x_mask[:, :])
        kio_t = const_pool.tile([P, 1], f32)
        nc.gpsimd.dma_start(kio_t[:], x_kio[:, :])
        # warm the ACT exp table now (1.3us table load otherwise lands on
        # the first real exp's critical path); scale=0 makes the
        # uninitialized input irrelevant
        warm_t = const_pool.tile([P, 1], bf16)
        nc.scalar.activation(warm_t[:], warm_t[:], EXP, scale=0.0)


        # per-head state created lazily by the flat tile stream
        vt_ts, vq_ts, o_bigs, pt_tiles = {}, {}, {}, {}

        # engine load balancer state (ns model)
        load = {"act": 0.0, "dve": 0.0}

        def load_head(h):
            # rebalance locally per head: a global tally lets one engine
            # run time-locally hot even when cumulative loads match
            m = min(load["act"], load["dve"])
            load["act"] -= m
            load["dve"] -= m
            vt_t = vt_pool.tile([P, L], bf16, tag="vt")
            vq_t = vq_pool.tile([P, NKB, D + 1], bf16, tag="vq")
            if h == 0:
                # cold start: land the first 256 columns on their own so
                # the first (split) scores matmul fires earlier
                nc.sync.dma_start(vt_t[:, 0:256], x_vt[h][0][:, 0:256])
                nc.sync.dma_start(vt_t[:, 256:512], x_vt[h][0][:, 256:512])
                for c in range(1, 4):
                    nc.sync.dma_start(vt_t[:, c * 512:(c + 1) * 512],
                                      x_vt[h][c])
            else:
                for c in range(4):  # vt first: the first scores tile needs it
                    nc.sync.dma_start(vt_t[:, c * 512:(c + 1) * 512],
                                      x_vt[h][c])
            for c in range(2):
                nc.sync.dma_start(
                    vq_t[:, c * QB:(c + 1) * QB, :],
                    x_vq[h][c].rearrange("p (kb c) -> p kb c", kb=QB),
                )
            vt_ts[h], vq_ts[h] = vt_t, vq_t
            o_bigs[h] = out_pool.tile([P, NKB, D], bf16, tag="obig",
                                      name="obig")

        pending_norms = []  # (h, qi) finished chains awaiting normalize

        def flush_norms():
            while pending_norms:
                h, qi, po = pending_norms.pop(0)
                nc.vector._custom_dve(
                    NORM_OP,
                    out=o_bigs[h][:, qi, :],
                    in0=po[:, :D],
                    s0=po[:, D:D + 1],
                    s1=NR_C0,
                    imm2=NR_C1,
                )
                load["dve"] += NORM_COST
                if h == H4 - 1 and qi <= 11:
                    # last head: drain early rows per-qi via Sync HWDGE
                    # (idle there) so only the last quarter remains at the
                    # end
                    nc.sync.dma_start(y[h][qi * P:(qi + 1) * P, :],
                                      o_bigs[h][:, qi, :])
                elif h == H4 - 1 and qi == 13:
                    # final quarter leaves in overlapping pieces on two DMA
                    # queues: rows 12-13 as soon as their norms land, then
                    # 14 and 15 in parallel on separate queues
                    nc.sync.dma_start(
                        y[h][1536:1792, :].rearrange("(kb p) d -> p kb d",
                                                     p=P),
                        o_bigs[h][:, 12:14, :],
                    )
                elif h == H4 - 1 and qi == 14:
                    nc.gpsimd.dma_start(y[h][1792:1920, :],
                                        o_bigs[h][:, 14, :])
                elif h == H4 - 1 and qi == 15:
                    nc.sync.dma_start(y[h][1920:2048, :],
                                      o_bigs[h][:, 15, :])
                elif qi % 4 == 3:  # finished an output quarter -> drain it
                    q4 = qi // 4
                    nc.gpsimd.dma_start(
                        y[h][q4 * 512:(q4 + 1) * 512, :].rearrange(
                            "(kb p) d -> p kb d", p=P),
                        o_bigs[h][:, q4 * 4:(q4 + 1) * 4, :],
                    )

        def emit_scores_tile(h, qc, kb):
            j = kb - QB * qc  # >=0 -> diagonal-chunk block
            off = max(0, j) * P
            vt_t = vt_ts[h]
            q0 = qc * QC
            ps = ps_s.tile([P, QC], f32, tag="ps_s")
            if off < 512:
                if h == 0 and qc == 0 and kb == 0:
                    # split the very first matmul so it only waits on the
                    # first half of the cold-start vt DMA
                    nc.tensor.matmul(ps[:, 0:256], vt_t[:, 0:P],
                                     vt_t[:, 0:256], start=True, stop=True)
                    nc.tensor.matmul(ps[:, 256:512], vt_t[:, 0:P],
                                     vt_t[:, 256:512], start=True, stop=True)
                else:
                    nc.tensor.matmul(ps[:, off:512],
                                     vt_t[:, kb * P:(kb + 1) * P],
                                     vt_t[:, q0 + off:q0 + 512],
                                     start=True, stop=True)
                nc.tensor.matmul(ps[:, 512:],
                                 vt_t[:, kb * P:(kb + 1) * P],
                                 vt_t[:, q0 + 512:q0 + QC],
                                 start=True, stop=True)
            else:
                nc.tensor.matmul(ps[:, off:],
                                 vt_t[:, kb * P:(kb + 1) * P],
                                 vt_t[:, q0 + off:q0 + QC],
                                 start=True, stop=True)
            pt = pt_pool.tile([P, QC], bf16, tag="pt")
            cols = QC - off
            cost_a = cols * ACT_RATE + ACT_OVH
            cost_d = cols * DVE_RATE + DVE_OVH
            # on the last head DVE still owns the epilogue norms while ACT
            # goes idle, so shift exp work toward ACT there
            bias = DVE_BIAS + (1600.0 if h == H4 - 1 else 0.0)
            use_act = load["act"] + cost_a <= load["dve"] + cost_d + bias
            if use_act:
                load["act"] += cost_a
                nc.scalar.activation(pt[:, off:], ps[:, off:], EXP,
                                     scale=SCALE)
                if j >= 0:
                    # mask on GPSIMD: keeps the DVE queue free for exp+norm
                    nc.gpsimd.tensor_mul(pt[:, off:off + P],
                                         pt[:, off:off + P], mask_t[:])
            else:
                load["dve"] += cost_d
                nc.vector._custom_dve(
                    SCHRA_OP,
                    out=pt[:, off:].bitcast(i16),
                    in0=ps[:, off:],
                    s0=(kio_t[:] if j >= 0 else 0.0),
                    s1=SA,
                    imm2=SB,
                )
            flush_norms()
            pt_tiles[(h, qc, kb)] = pt

        # PV chain work is drained as individual matmuls from a FIFO so
        # each scores tile is followed by just enough PV matmuls to fill
        # PE's slack while the exp engines run. A backlog floor keeps work
        # in reserve for the chain-less pass-1-early windows.
        chain_fifo = []   # (h, qc, qi) in completion order
        cur = {"mm": 0, "po": None}   # cursor into chain_fifo[0]
        backlog = {"mms": 0}
        done = {"scores": False}

        def emit_chain_mms(n):
            while n > 0 and chain_fifo:
                h, qc, qi = chain_fifo[0]
                qsub = qi - QB * qc
                if cur["po"] is None:
                    cur["po"] = ps_o.tile([P, D + 1], f32, tag="ps_o",
                                          name="po")
                    cur["mm"] = 0
                kb = cur["mm"]
                nc.tensor.matmul(
                    cur["po"][:],
                    pt_tiles[(h, qc, kb)][:, qsub * P:(qsub + 1) * P],
                    vq_ts[h][:, kb, :],
                    start=(kb == 0), stop=(kb == qi),
                )
                cur["mm"] += 1
                backlog["mms"] -= 1
                n -= 1
                if cur["mm"] == qi + 1:
                    pending_norms.append((h, qi, cur["po"]))
                    chain_fifo.pop(0)
                    cur["po"] = None
                    # in the epilogue there are no more scores tiles to
                    # piggyback on: emit norms as chains complete so the
                    # output drains overlap the remaining PV matmuls
                    if done["scores"]:
                        flush_norms()

        # chain matmuls run ~45% faster in uninterrupted streaks (weight
        # loads pipeline), so accumulate the per-tile budget and emit in
        # bursts of ~2 tiles' worth
        FLOOR = 56
        BURST = 12
        budget_acc = {"n": 0}
        for h in range(H4):
            load_head(h)
            for qc in range(NQC):
                for kb in range(QB * qc + QB):
                    j = kb - QB * qc
                    # the reserve exists FOR the chain-less pass-1-early
                    # window: release it there (and on the last head),
                    # hold it during append windows
                    in_p1_early = qc == NQC - 1 and j < 0
                    floor = 0 if (h == H4 - 1 or in_p1_early) else FLOOR
                    emit_scores_tile(h, qc, kb)
                    if j >= 0:
                        qi = QB * qc + j
                        chain_fifo.append((h, qc, qi))
                        backlog["mms"] += qi + 1
                    cols = QC - max(0, j) * P
                    budget_acc["n"] += cols // 190 + 2
                    if h == H4 - 1 and qc == NQC - 1:
                        # drain the backlog during the final chunk so the
                        # last chains don't trail the last scores tile
                        budget_acc["n"] += 12
                    if budget_acc["n"] >= BURST:
                        emit_chain_mms(
                            min(budget_acc["n"], backlog["mms"] - floor))
                        budget_acc["n"] = 0
        done["scores"] = True
        emit_chain_mms(backlog["mms"])
        flush_norms()

    nc.compile()
    return nc


def _get_nc():
    if "nc" not in _cache:
        _cache["nc"] = _build_nc()
    return _cache["nc"]


def _make_mask():
    # keep (partition=k_local, free=q_local) where q_local >= k_local
    pk = np.arange(P)[:, None]
    fq = np.arange(P)[None, :]
    return (fq >= pk).astype(BF16)


def kernel(x):
    from concourse.bass_utils import run_bass_kernel_spmd

    x = np.asarray(x)
    in_dtype = x.dtype
    assert x.shape == (B, L, E)

    nc = _get_nc()

    # (B, L, H, D) -> (B*H, L, D), bf16
    v = np.ascontiguousarray(
        x.reshape(B, L, H, D).transpose(0, 2, 1, 3)
    ).reshape(B * H, L, D).astype(BF16)

    mask = _make_mask()
    kio = np.arange(P, dtype=np.float32).reshape(P, 1)
    in_maps = []
    for c in range(NCORES):
        sl = v[H4 * c:H4 * (c + 1)]                      # (H4, L, D)
        # chunk-major vq: [H4, 2, P, QB*(D+1)], ones column appended
        vq = np.ones((H4, P, NKB, D + 1), dtype=BF16)
        vq[..., :D] = sl.reshape(H4, NKB, P, D).transpose(0, 2, 1, 3)
        vq = np.ascontiguousarray(
            vq.reshape(H4, P, 2, QB * (D + 1)).transpose(0, 2, 1, 3))
        # chunk-major vt: [H4, 4, D, 512]
        vt = sl.transpose(0, 2, 1).reshape(H4, D, 4, 512)
        vt = np.ascontiguousarray(vt.transpose(0, 2, 1, 3))
        in_maps.append({"vq": vq, "vt": vt, "mask": mask, "kio": kio})

    import os

    kwargs = {}
    if os.environ.get("KERNEL_TRACE"):
        kwargs["trace"] = True
        if os.environ.get("KERNEL_TRACE_DIR"):
            kwargs["tmpdir"] = os.environ["KERNEL_TRACE_DIR"]
    res = run_bass_kernel_spmd(nc, in_maps, core_ids=list(range(NCORES)), **kwargs)
    _cache["last_results"] = res
    ys = np.stack([np.asarray(res.results[c]["y"]) for c in range(NCORES)],
                  axis=0)
    # (NCORES, H4, L, D) -> (B, H, L, D) -> (B, L, E)
    out = ys.astype(np.float32).reshape(B, H, L, D).transpose(
        0, 2, 1, 3).reshape(B, L, E)
    return out.astype(in_dtype, copy=False)
